# revision 12
# baseline (speedup 1.0000x reference)
# Trainium2 Bass kernel for DiffeomorphicNetwork (scaling-and-squaring warp + TopoFix).
#
# Key algorithmic fact: |flow_k|_inf <= 1.5 * 2^k / 64, so every squaring step
# displaces < 1 voxel (3-tap stencil reach) and the final mask warp < 2 voxels
# (5-tap reach). Trilinear gather therefore becomes a dense separable stencil
# with per-voxel hat weights  hat(f - o) = max(0, 1 - |f - o|); clamping the
# fractional flow to the global volume bounds makes out-of-range tap weights
# exactly zero, which reproduces border (clamp) sampling semantics exactly.
#
# Sharding: data-parallel over (batch, z-quarter) across 8 cores, zero
# collectives. Each core computes its 32 output slices from a 44-slice halo
# window; halo validity shrinks by one slice per squaring step.
#
# Device layout: partition = y (128 rows), free dim = flat run of z-blocks of
# 132 columns ([2 pad | 128 x | 2 pad]). x/z shifts are free-dim offsets;
# y shifts (and x pre-shifts, for aligned bf16 2x mode) are produced during
# DMA loads from a flat DRAM flow scratch that ping-pongs between steps.
# The three flow channels are stacked in one (128, 3, width) tile per variant
# so each DVE instruction processes all channels (weights broadcast stride-0).

import numpy as np

import concourse.bacc as bacc
import concourse.mybir as mybir
import concourse.tile as tile
from concourse.bass_interp import get_hw_module

F32 = mybir.dt.float32
BF16 = mybir.dt.bfloat16
AF = mybir.ActivationFunctionType
OP = mybir.AluOpType

P = 128          # partitions = y
W = 128          # x extent
PAD = 2
BLK = W + 2 * PAD  # 132 columns per z-block
NB = 44          # flow window blocks per core (32 out + 2*6 halo)
NXM = 36         # x-mask blocks per core (32 out + 2*2 halo)
NOUT = 32
NSTEP = 6
VSCALE = 1.5 / (2.0 ** NSTEP)
EPS = 1e-4
GN = 4           # z-blocks per compute group (exact squaring steps)
GN_FO = 8        # z-blocks per group (first-order squaring steps)
N_FO = 4         # leading steps using the first-order star stencil
GNF = 6          # z-blocks per group (final 5-tap warp)
N_CORES = 8
FLAT = 2 + NB * BLK + 2   # flat flow row length (lead/trail pads)
XMPAD = BLK + 4           # x-mask flat lead/trail pad (one dummy block + slack)

B, D, H, WW = 2, 128, 128, 128


class _Builder:
    _dma_rr = 0

    def _dma(self):
        # Alternate between the two physical HWDGE rings (qSPDynamicHW via
        # the sync engine, qActDynamicHW via the scalar engine) — a single
        # ring caps effective DMA bandwidth well below fabric rate.
        self._dma_rr ^= 1
        return self.nc.sync if self._dma_rr else self.nc.scalar

    def __init__(self, taps_dtype=BF16, loop_steps=0, loop_final=0, n_fo=N_FO,
                 skip_steps=False, skip_final=False):
        # loop_steps/loop_final=R>0 wrap that phase in a hardware For_i loop
        # (R iterations) for wall-clock timing builds. skip_* omit a phase
        # entirely (cost-model phase isolation; TimelineSim can't branch).
        self.loop_steps = loop_steps
        self.loop_final = loop_final
        self.skip_steps = skip_steps
        self.skip_final = skip_final
        self.n_fo = n_fo
        self.DT = taps_dtype
        nc = bacc.Bacc(
            "TRN2", target_bir_lowering=False, debug=False, enable_asserts=False
        )
        self.nc = nc
        self.rawv = nc.dram_tensor("rawv", (3, P, NB * BLK), F32, kind="ExternalInput").ap()
        self.xm = nc.dram_tensor("xm", (P, NXM * BLK), F32, kind="ExternalInput").ap()
        self.rawt = nc.dram_tensor("rawt", (P, NOUT * W), F32, kind="ExternalInput").ap()
        self.zb = nc.dram_tensor("zb", (P, 2, 64), F32, kind="ExternalInput").ap()
        self.yb = nc.dram_tensor("yb", (P, 2), F32, kind="ExternalInput").ap()
        self.xb = nc.dram_tensor("xb", (P, 2, BLK), F32, kind="ExternalInput").ap()
        self.out = nc.dram_tensor("out", (P, NOUT * W), F32, kind="ExternalOutput").ap()
        self.flowA = nc.dram_tensor("flowA", (3, P, FLAT), self.DT, kind="Internal").ap()
        self.flowB = nc.dram_tensor("flowB", (3, P, FLAT), self.DT, kind="Internal").ap()
        self.xmf = nc.dram_tensor("xmf", (P, XMPAD + NXM * BLK + XMPAD), self.DT, kind="Internal").ap()

    def build(self):
        nc = self.nc
        DT = self.DT
        with tile.TileContext(nc) as tc:
            with tc.tile_pool(name="const", bufs=1) as cpool:
                self.tc = tc
                self.cpool = cpool
                for val in (2.0, -1.0, -2.0, EPS, 1.0 + EPS):
                    t = nc.alloc_sbuf_tensor(f"constx-{val}", [P, 1], F32)
                    nc.gpsimd.memset(t.ap(), val)
                    nc.const_aps.aps[(F32, val)] = t.ap()
                self.yb_s = cpool.tile([P, 2], F32, tag="yb")
                nc.sync.dma_start(self.yb_s[:], self.yb)
                xb_f = cpool.tile([P, 2, BLK], F32, tag="xbf")
                nc.sync.dma_start(xb_f[:], self.xb)
                self.xb_s = cpool.tile([P, 2, BLK], DT, tag="xb")
                nc.vector.tensor_copy(self.xb_s[:], xb_f[:])
                self.zlo_t, self.zhi_t = [], []
                for j in range(NB):
                    lo = cpool.tile([P, 1], F32, tag=f"zlo{j}")
                    nc.sync.dma_start(lo[:], self.zb[:, 0, j : j + 1].squeeze(1))
                    hi = cpool.tile([P, 1], F32, tag=f"zhi{j}")
                    nc.sync.dma_start(hi[:], self.zb[:, 1, j : j + 1].squeeze(1))
                    self.zlo_t.append(lo)
                    self.zhi_t.append(hi)
                # zero the lead/trail pad columns of the flat scratch rows
                zt = cpool.tile([P, 3, 2], DT, tag="zpad")
                nc.vector.memset(zt[:], 0.0)
                for f in (self.flowA, self.flowB):
                    nc.sync.dma_start(f[:, :, 0:2].rearrange("c p w -> p c w"), zt[:])
                    nc.sync.dma_start(
                        f[:, :, FLAT - 2 : FLAT].rearrange("c p w -> p c w"), zt[:]
                    )
                zx = cpool.tile([P, XMPAD], DT, tag="zxpad")
                nc.vector.memset(zx[:], 0.0)
                nc.sync.dma_start(self.xmf[:, 0:XMPAD], zx[:])
                nc.sync.dma_start(self.xmf[:, XMPAD + NXM * BLK :], zx[:])

                with tc.tile_pool(name="prep", bufs=1) as ppool:
                    self.bpool = ppool
                    self._prep()

                def _all_steps():
                    bufs = [self.flowA, self.flowB]
                    if self.n_fo:
                        with tc.tile_pool(name="obig", bufs=1) as bpool, \
                             tc.tile_pool(name="owt", bufs=1) as wpool, \
                             tc.tile_pool(name="otmp", bufs=1) as tpool:
                            self.bpool, self.wpool, self.tpool = bpool, wpool, tpool
                            for k in range(self.n_fo):
                                fin, fout = bufs[k % 2], bufs[(k + 1) % 2]
                                lo, hi = k + 1, NB - (k + 1)
                                g0 = lo
                                while g0 < hi:
                                    gn = min(GN_FO, hi - g0)
                                    self._step_group_fo(fin, fout, g0, gn)
                                    g0 += gn
                    if self.n_fo < NSTEP:
                        with tc.tile_pool(name="sbig", bufs=1) as bpool, \
                             tc.tile_pool(name="swt", bufs=1) as wpool, \
                             tc.tile_pool(name="stmp", bufs=1) as tpool:
                            self.bpool, self.wpool, self.tpool = bpool, wpool, tpool
                            for k in range(self.n_fo, NSTEP):
                                fin, fout = bufs[k % 2], bufs[(k + 1) % 2]
                                lo, hi = k + 1, NB - (k + 1)
                                g0 = lo
                                while g0 < hi:
                                    gn = min(GN, hi - g0)
                                    self._step_group(fin, fout, g0, gn)
                                    g0 += gn

                if self.skip_steps:
                    pass
                elif self.loop_steps:
                    with tc.For_i(0, self.loop_steps, 1):
                        _all_steps()
                else:
                    _all_steps()
                with tc.tile_pool(name="fbig", bufs=1) as bpool, \
                     tc.tile_pool(name="fwt", bufs=1) as wpool, \
                     tc.tile_pool(name="ftmp", bufs=1) as tpool:
                    self.bpool, self.wpool, self.tpool = bpool, wpool, tpool

                    def _all_final():
                        # after 6 steps flow_6 is in flowA
                        g0 = 6
                        while g0 < 38:
                            gn = min(GNF, 38 - g0)
                            self._final_group(self.flowA, g0, gn)
                            g0 += gn

                    if self.skip_final:
                        pass
                    elif self.loop_final:
                        with tc.For_i(0, self.loop_final, 1):
                            _all_final()
                    else:
                        _all_final()
        nc.compile()
        nc.m = get_hw_module(nc.m)
        return nc

    # ---------- helpers ----------
    def _prep(self):
        """flow_0 = tanh(rawv) * VSCALE -> flowA; x-mask -> xmf (cast to DT)."""
        nc = self.nc
        for g0 in range(0, NB, 8):
            gn = min(8, NB - g0)
            t = self.bpool.tile([P, 3, gn, BLK], F32, tag="prepf", bufs=2)
            for c in range(3):
                self._dma().dma_start(
                    t[:, c],
                    self.rawv[c][:, g0 * BLK : (g0 + gn) * BLK].rearrange(
                        "p (z x) -> p z x", x=BLK
                    ),
                )
            t2 = self.bpool.tile([P, 3, gn, BLK], self.DT, tag="prepb", bufs=2)
            nc.scalar.activation(t2[:], t[:], AF.Tanh)
            t3 = self.bpool.tile([P, 3, gn, BLK], self.DT, tag="prepc", bufs=2)
            nc.scalar.mul(t3[:], t2[:], VSCALE)
            for c in range(3):
                self._dma().dma_start(
                    self.flowA[c, :, 2 + g0 * BLK : 2 + (g0 + gn) * BLK].rearrange(
                        "p (z x) -> p z x", x=BLK
                    ),
                    t3[:, c],
                )
        for g0 in range(0, NXM, 12):
            gn = min(12, NXM - g0)
            t = self.bpool.tile([P, gn, BLK], F32, tag="prepx", bufs=2)
            self._dma().dma_start(
                t[:],
                self.xm[:, g0 * BLK : (g0 + gn) * BLK].rearrange(
                    "p (z x) -> p z x", x=BLK
                ),
            )
            t2 = self.bpool.tile([P, gn, BLK], self.DT, tag="prepy", bufs=2)
            nc.vector.tensor_copy(t2[:], t[:])
            nc.sync.dma_start(
                self.xmf[:, XMPAD + g0 * BLK : XMPAD + (g0 + gn) * BLK].rearrange(
                    "p (z x) -> p z x", x=BLK
                ),
                t2[:],
            )

    def _load_var(self, fin, off, extw, oy, ox, tag, bufs=2):
        """Load stacked (P,3,extw) window at flat col off+ox with y-shift oy."""
        nc = self.nc
        t = self.bpool.tile([P, 3, extw], self.DT, tag=tag, bufs=bufs)
        sl = fin[:, :, off + ox : off + ox + extw]
        if oy == 0:
            self._dma().dma_start(t[:], sl.rearrange("c p w -> p c w"))
        elif oy > 0:
            self._dma().dma_start(
                t[0 : P - oy], sl[:, oy:P].rearrange("c p w -> p c w")
            )
            for r in range(P - oy, P):
                nc.sync.dma_start(
                    t[r : r + 1], sl[:, P - 1 : P].rearrange("c p w -> p c w")
                )
        else:
            s = -oy
            self._dma().dma_start(
                t[s:P], sl[:, 0 : P - s].rearrange("c p w -> p c w")
            )
            for r in range(s):
                nc.sync.dma_start(
                    t[r : r + 1], sl[:, 0:1].rearrange("c p w -> p c w")
                )
        return t

    def _clamp_weights(self, V0, gn, g0, base):
        """Clamped fields from the oy=0,ox=0 stacked variant.
        base = col offset of the out window inside the variant tiles."""
        nc = self.nc
        DT = self.DT
        gw = gn * BLK
        fx = V0[:, 0, base : base + gw]
        fy = V0[:, 1, base : base + gw]
        fz = V0[:, 2, base : base + gw]
        fxc = self.wpool.tile([P, gw], DT, tag="fxc", bufs=2)
        xhi = self.xb_s[:, 1, :].unsqueeze(1).broadcast_to((P, gn, BLK))
        xlo = self.xb_s[:, 0, :].unsqueeze(1).broadcast_to((P, gn, BLK))
        v3 = fxc[:].rearrange("p (z x) -> p z x", x=BLK)
        nc.vector.tensor_tensor(v3, fx.rearrange("p (z x) -> p z x", x=BLK), xhi, OP.min)
        nc.vector.tensor_tensor(v3, v3, xlo, OP.max)
        fyc = self.wpool.tile([P, gw], DT, tag="fyc", bufs=2)
        nc.vector.tensor_scalar(
            fyc[:], fy, self.yb_s[:, 1:2], self.yb_s[:, 0:1], OP.min, OP.max
        )
        fzc = self.wpool.tile([P, gw], DT, tag="fzc", bufs=2)
        for b in range(gn):
            nc.vector.tensor_scalar(
                fzc[:, b * BLK : (b + 1) * BLK],
                fz[:, b * BLK : (b + 1) * BLK],
                self.zhi_t[g0 + b][:],
                self.zlo_t[g0 + b][:],
                OP.min,
                OP.max,
            )
        return fxc, fyc, fzc

    def _hat3(self, fc, gw, pref):
        nc = self.nc
        DT = self.DT
        hp = self.wpool.tile([P, gw], DT, tag=pref + "p", bufs=2)
        nc.scalar.activation(hp[:], fc[:], AF.Relu)
        hm = self.wpool.tile([P, gw], DT, tag=pref + "m", bufs=2)
        nc.scalar.activation(hm[:], fc[:], AF.Relu, scale=-1.0)
        ab = self.wpool.tile([P, gw], DT, tag="hab", bufs=2)
        nc.scalar.activation(ab[:], fc[:], AF.Abs)
        h0 = self.wpool.tile([P, gw], DT, tag=pref + "0", bufs=2)
        nc.vector.tensor_scalar(h0[:], ab[:], -1.0, 1.0, OP.mult, OP.add)
        return hm, h0, hp

    def _bc3(self, wt, w0, w1):
        return wt[:, w0:w1].unsqueeze(1).broadcast_to((P, 3, w1 - w0))

    def _step_group(self, fin, fout, g0, gn):
        nc = self.nc
        DT = self.DT
        ext = gn + 2
        extw = ext * BLK
        e0 = g0 - 1
        off = 2 + e0 * BLK
        gw = gn * BLK
        # Load only the 3 y-shifted stacked variants (DMA descriptors are the
        # HW bottleneck); derive the y-difference fields on DVE and all six
        # x-shifted variants via ScalarE copies (ACT has no alignment
        # constraint, so its outputs are fresh aligned tiles and every DVE
        # app op keeps bf16 2x mode). The one column per x-shifted variant
        # that falls outside the on-chip window only ever pairs with
        # weight-exactly-zero pad positions, so it is memset instead.
        Vb = {}
        for oy in (-1, 0, 1):
            Vb[oy] = self._load_var(fin, off, extw, oy, 0, f"V{oy}0", bufs=2)
        ey0 = self.bpool.tile([P, 3, extw], DT, tag="Ey0", bufs=2)
        nc.vector.tensor_tensor(ey0[:], Vb[1][:], Vb[0][:], OP.subtract)
        gy0 = self.bpool.tile([P, 3, extw], DT, tag="Gy0", bufs=2)
        nc.vector.tensor_tensor(gy0[:], Vb[-1][:], Vb[0][:], OP.subtract)

        def xshift(srct, d, tag):
            t = self.bpool.tile([P, 3, extw], DT, tag=tag, bufs=2)
            if d > 0:
                nc.scalar.copy(t[:, :, 0 : extw - d], srct[:, :, d:extw])
                nc.vector.memset(t[:, :, extw - d : extw], 0.0)
            else:
                nc.scalar.copy(t[:, :, -d : extw], srct[:, :, 0 : extw + d])
                nc.vector.memset(t[:, :, 0:-d], 0.0)
            return t

        Vv = {(0, 0): Vb[0]}
        Ey, Gy = {0: ey0}, {0: gy0}
        for d in (-1, 1):
            Vv[(0, d)] = xshift(Vb[0], d, f"Vx{d}")
            Ey[d] = xshift(ey0, d, f"Eyx{d}")
            Gy[d] = xshift(gy0, d, f"Gyx{d}")

        fxc, fyc, fzc = self._clamp_weights(Vv[(0, 0)], gn, g0, BLK)
        fpy = self.wpool.tile([P, gw], DT, tag="fpy", bufs=2)
        nc.scalar.activation(fpy[:], fyc[:], AF.Relu)
        fmy = self.wpool.tile([P, gw], DT, tag="fmy", bufs=2)
        nc.scalar.activation(fmy[:], fyc[:], AF.Relu, scale=-1.0)
        hxm, hx0, hxp = self._hat3(fxc, gw, "hx")
        hzm, hz0, hzp = self._hat3(fzc, gw, "hz")

        I = {}
        for oz in (-1, 0, 1):
            bo = (1 + oz) * BLK  # window start inside the variant tiles
            T = {}
            for ox in (-1, 0, 1):
                t = self.tpool.tile([P, 3, gw], DT, tag="T", bufs=3)
                m = self.tpool.tile([P, 3, gw], DT, tag="m", bufs=2)
                vs = slice(bo, bo + gw)
                nc.vector.tensor_tensor(
                    m[:], self._bc3(fpy, 0, gw), Ey[ox][:, :, vs], OP.mult
                )
                nc.vector.tensor_tensor(t[:], m[:], Vv[(0, ox)][:, :, vs], OP.add)
                nc.vector.tensor_tensor(
                    m[:], self._bc3(fmy, 0, gw), Gy[ox][:, :, vs], OP.mult
                )
                nc.vector.tensor_tensor(t[:], t[:], m[:], OP.add)
                T[ox] = t
            it = self.tpool.tile([P, 3, gw], DT, tag="I", bufs=3)
            nc.vector.tensor_tensor(it[:], self._bc3(hx0, 0, gw), T[0][:], OP.mult)
            m = self.tpool.tile([P, 3, gw], DT, tag="m", bufs=2)
            nc.vector.tensor_tensor(m[:], self._bc3(hxm, 0, gw), T[-1][:], OP.mult)
            nc.vector.tensor_tensor(it[:], it[:], m[:], OP.add)
            nc.vector.tensor_tensor(m[:], self._bc3(hxp, 0, gw), T[1][:], OP.mult)
            nc.vector.tensor_tensor(it[:], it[:], m[:], OP.add)
            I[oz] = it
        ot = self.tpool.tile([P, 3, gw], DT, tag="O", bufs=2)
        nc.vector.tensor_tensor(ot[:], self._bc3(hzm, 0, gw), I[-1][:], OP.mult)
        nc.vector.tensor_tensor(
            ot[:], ot[:], Vv[(0, 0)][:, :, BLK : BLK + gw], OP.add
        )
        m = self.tpool.tile([P, 3, gw], DT, tag="m", bufs=2)
        nc.vector.tensor_tensor(m[:], self._bc3(hz0, 0, gw), I[0][:], OP.mult)
        nc.vector.tensor_tensor(ot[:], ot[:], m[:], OP.add)
        nc.vector.tensor_tensor(m[:], self._bc3(hzp, 0, gw), I[1][:], OP.mult)
        nc.vector.tensor_tensor(ot[:], ot[:], m[:], OP.add)
        o4 = ot[:].rearrange("p c (z x) -> p c z x", x=BLK)
        nc.vector.memset(o4[:, :, :, 0:PAD], 0.0)
        nc.vector.memset(o4[:, :, :, W + PAD : BLK], 0.0)
        self._dma().dma_start(
            fout[:, :, 2 + g0 * BLK : 2 + g0 * BLK + gw].rearrange("c p w -> p c w"),
            ot[:],
        )

    def _step_group_fo(self, fin, fout, g0, gn):
        """First-order (star-stencil) squaring step for tiny displacements.

        warp(V, f) ~= V + sum_axis [ relu(fa)*(V(+a)-V) + relu(-fa)*(V(-a)-V) ]
        is exact per axis and drops only O(f^2) cross terms; for steps k < N_FO
        |flow_k| <= 1.5*2^k/64 <= 0.1875 voxels so the dropped terms are far
        below the error budget (host-validated: adds 2.1e-3 rel for N_FO=4...
        steps 0-3). Needs only the oy=0 window extended (for z taps) plus the
        two y-shifted variants on the center window, and 18 DVE passes per
        group instead of ~60.
        """
        nc = self.nc
        DT = self.DT
        ext = gn + 2
        extw = ext * BLK
        e0 = g0 - 1
        off = 2 + e0 * BLK
        gw = gn * BLK
        V0 = self._load_var(fin, off, extw, 0, 0, "V0f", bufs=2)
        Vyp = self._load_var(fin, off + BLK, gw, 1, 0, "Vypf", bufs=2)
        Vym = self._load_var(fin, off + BLK, gw, -1, 0, "Vymf", bufs=2)
        V0c = V0[:, :, BLK : BLK + gw]
        # x taps via ScalarE copies (no alignment constraint -> DVE keeps 2x)
        Vxp = self.bpool.tile([P, 3, gw], DT, tag="Vxpf", bufs=2)
        nc.scalar.copy(Vxp[:], V0[:, :, BLK + 1 : BLK + 1 + gw])
        Vxm = self.bpool.tile([P, 3, gw], DT, tag="Vxmf", bufs=2)
        nc.scalar.copy(Vxm[:], V0[:, :, BLK - 1 : BLK - 1 + gw])
        fxc, fyc, fzc = self._clamp_weights(V0, gn, g0, BLK)
        wts = {}
        ab = {}
        for nm, fc in (("x", fxc), ("y", fyc), ("z", fzc)):
            p = self.wpool.tile([P, gw], DT, tag=f"fp{nm}f", bufs=2)
            nc.scalar.activation(p[:], fc[:], AF.Relu)
            mw = self.wpool.tile([P, gw], DT, tag=f"fm{nm}f", bufs=2)
            nc.scalar.activation(mw[:], fc[:], AF.Relu, scale=-1.0)
            a = self.wpool.tile([P, gw], DT, tag=f"ab{nm}f", bufs=2)
            nc.scalar.activation(a[:], fc[:], AF.Abs)
            wts[nm] = (p, mw)
            ab[nm] = a
        # flow' = 2*V0 + sum_t w_t*(V_t - V0) = sum_t w_t*V_t + (2 - sum|f|)*V0
        w0 = self.wpool.tile([P, gw], DT, tag="w0f", bufs=2)
        nc.vector.tensor_tensor(w0[:], ab["x"][:], ab["y"][:], OP.add)
        nc.vector.tensor_tensor(w0[:], w0[:], ab["z"][:], OP.add)
        nc.vector.tensor_scalar(w0[:], w0[:], -1.0, 2.0, OP.mult, OP.add)
        acc = self.tpool.tile([P, 3, gw], DT, tag="accf", bufs=2)
        m = self.tpool.tile([P, 3, gw], DT, tag="mf", bufs=2)
        nc.vector.tensor_tensor(acc[:], self._bc3(wts["y"][0], 0, gw), Vyp[:], OP.mult)
        taps = (
            (wts["y"][1], Vym[:]),
            (wts["x"][0], Vxp[:]),
            (wts["x"][1], Vxm[:]),
            (wts["z"][0], V0[:, :, 2 * BLK : 2 * BLK + gw]),
            (wts["z"][1], V0[:, :, 0:gw]),
            (w0, V0c),
        )
        for wt, v in taps:
            nc.vector.tensor_tensor(m[:], self._bc3(wt, 0, gw), v, OP.mult)
            nc.vector.tensor_tensor(acc[:], acc[:], m[:], OP.add)
        o4 = acc[:].rearrange("p c (z x) -> p c z x", x=BLK)
        nc.vector.memset(o4[:, :, :, 0:PAD], 0.0)
        nc.vector.memset(o4[:, :, :, W + PAD : BLK], 0.0)
        self._dma().dma_start(
            fout[:, :, 2 + g0 * BLK : 2 + g0 * BLK + gw].rearrange("c p w -> p c w"),
            acc[:],
        )

    def _load_xvar(self, off, extw, oy, ox, tag):
        nc = self.nc
        t = self.bpool.tile([P, extw], self.DT, tag=tag, bufs=2)
        sl = self.xmf[:, off + ox : off + ox + extw]
        if oy == 0:
            self._dma().dma_start(t[:], sl)
        elif oy > 0:
            self._dma().dma_start(t[0 : P - oy], sl[oy:P])
            for r in range(P - oy, P):
                nc.sync.dma_start(t[r : r + 1], sl[P - 1 : P])
        else:
            s = -oy
            self._dma().dma_start(t[s:P], sl[0 : P - s])
            for r in range(s):
                nc.sync.dma_start(t[r : r + 1], sl[0:1])
        return t

    def _hat5(self, fc, gw, pref):
        nc = self.nc
        hats = {}
        for o in (-2, -1, 0, 1, 2):
            ab = self.wpool.tile([P, gw], self.DT, tag="hab", bufs=2)
            nc.scalar.activation(ab[:], fc[:], AF.Abs, bias=float(-o))
            h = self.wpool.tile([P, gw], self.DT, tag=f"{pref}h{o}", bufs=1)
            nc.scalar.activation(h[:], ab[:], AF.Relu, scale=-1.0, bias=1.0)
            hats[o] = h
        return hats

    def _final_group(self, fin, g0, gn):
        """Out blocks local z in [g0, g0+gn); 5-tap warp of x-mask + TopoFix."""
        nc = self.nc
        DT = self.DT
        ext = gn + 6
        extw = ext * BLK
        e0 = g0 - 7          # xm flat block index (xm idx i <-> local z i+4)
        off = XMPAD + e0 * BLK
        gw = gn * BLK
        # Load only the 5 y-shifted variants; synthesize the x-shifted ones
        # with ScalarE copies (no alignment constraint, outputs are aligned
        # tiles, and DMA descriptors were the HW bottleneck). The one
        # unwritten edge column per synthesized tile is never read: app
        # windows span [BLK, 5*BLK+gw) which stays >=1 column inside.
        XV = {}
        for oy in (-2, -1, 0, 1, 2):
            XV[(oy, 0)] = self._load_xvar(off, extw, oy, 0, f"XV{oy}0")
        for oy in (-2, -1, 0, 1, 2):
            for d in (-1, 1):
                t = self.bpool.tile([P, extw], DT, tag=f"XV{oy}{d}", bufs=1)
                if d > 0:
                    nc.scalar.copy(t[:, 0 : extw - 1], XV[(oy, 0)][:, 1:extw])
                else:
                    nc.scalar.copy(t[:, 1:extw], XV[(oy, 0)][:, 0 : extw - 1])
                XV[(oy, d)] = t

        F = self.bpool.tile([P, 3, gw], DT, tag="Fw", bufs=2)
        self._dma().dma_start(
            F[:],
            fin[:, :, 2 + g0 * BLK : 2 + g0 * BLK + gw].rearrange("c p w -> p c w"),
        )
        fxc, fyc, fzc = self._clamp_weights(F, gn, g0, 0)
        hx = self._hat5(fxc, gw, "kx")
        hy = self._hat5(fyc, gw, "ky")
        hz = self._hat5(fzc, gw, "kz")

        # oz-batched stages: each (P, 5, gw) tile holds the five oz windows as
        # overlapping stride-BLK views of one variant (rows share data), so a
        # single DVE op covers all five combos -> 5x the FD per instruction.
        def oz5(xv, xoff):
            v = xv[:, BLK + xoff : BLK + xoff + gw].unsqueeze(1).broadcast_to(
                (P, 5, gw)
            )
            v.ap[1] = [BLK, 5]
            return v

        def w5(wt):
            return wt[:].unsqueeze(1).broadcast_to((P, 5, gw))

        m5 = self.tpool.tile([P, 5, gw], DT, tag="m5", bufs=1)
        T = {}
        for ox in (-2, -1, 0, 1, 2):
            # use the pre-shifted +-1 loads; +-2 via aligned offset of the
            # ox=0 variant (stays 4B-aligned in bf16)
            if ox in (-1, 0, 1):
                xsrc, xoff = ox, 0
            else:
                xsrc, xoff = 0, ox
            t = self.tpool.tile([P, 5, gw], DT, tag=f"T{ox}", bufs=1)
            nc.vector.tensor_tensor(t[:], w5(hy[0]), oz5(XV[(0, xsrc)], xoff), OP.mult)
            for oy in (-2, -1, 1, 2):
                nc.vector.tensor_tensor(
                    m5[:], w5(hy[oy]), oz5(XV[(oy, xsrc)], xoff), OP.mult
                )
                nc.vector.tensor_tensor(t[:], t[:], m5[:], OP.add)
            T[ox] = t
        it5 = self.tpool.tile([P, 5, gw], DT, tag="I5", bufs=1)
        nc.vector.tensor_tensor(it5[:], w5(hx[0]), T[0][:], OP.mult)
        for ox in (-2, -1, 1, 2):
            nc.vector.tensor_tensor(m5[:], w5(hx[ox]), T[ox][:], OP.mult)
            nc.vector.tensor_tensor(it5[:], it5[:], m5[:], OP.add)
        wv = self.tpool.tile([P, gw], DT, tag="O", bufs=2)
        m = self.tpool.tile([P, gw], DT, tag="m", bufs=2)
        nc.vector.tensor_tensor(wv[:], hz[0][:], it5[:, 2, :], OP.mult)
        for oz in (-2, -1, 1, 2):
            nc.vector.tensor_tensor(m[:], hz[oz][:], it5[:, 2 + oz, :], OP.mult)
            nc.vector.tensor_tensor(wv[:], wv[:], m[:], OP.add)

        # TopoFix: sigmoid(log(w+eps) - log(1-w+eps) + tanh(rt)*sigmoid(rt))
        l1 = self.tpool.tile([P, gw], F32, tag="l1", bufs=1)
        nc.scalar.activation(l1[:], wv[:], AF.Ln, bias=EPS)
        l2 = self.tpool.tile([P, gw], F32, tag="l2", bufs=1)
        nc.scalar.activation(l2[:], wv[:], AF.Ln, scale=-1.0, bias=1.0 + EPS)
        nc.vector.tensor_tensor(l1[:], l1[:], l2[:], OP.subtract)

        gwr = gn * W
        rt = self.bpool.tile([P, gn, W], F32, tag="rt", bufs=2)
        self._dma().dma_start(
            rt[:],
            self.rawt[:, (g0 - 6) * W : (g0 - 6 + gn) * W].rearrange(
                "p (z x) -> p z x", x=W
            ),
        )
        rtf = rt[:].rearrange("p z x -> p (z x)")
        sg = self.tpool.tile([P, gwr], F32, tag="sg", bufs=1)
        nc.scalar.activation(sg[:], rtf, AF.Sigmoid)
        th = self.tpool.tile([P, gwr], F32, tag="th", bufs=1)
        nc.scalar.activation(th[:], rtf, AF.Tanh)
        nc.vector.tensor_tensor(sg[:], sg[:], th[:], OP.mult)
        l1v = l1[:].rearrange("p (z x) -> p z x", x=BLK)[:, :, PAD : PAD + W]
        res = self.tpool.tile([P, gn, W], F32, tag="res", bufs=2)
        nc.vector.tensor_tensor(
            res[:], l1v, sg[:].rearrange("p (z x) -> p z x", x=W), OP.add
        )
        nc.scalar.activation(res[:], res[:], AF.Sigmoid)
        self._dma().dma_start(
            self.out[:, (g0 - 6) * W : (g0 - 6 + gn) * W].rearrange(
                "p (z x) -> p z x", x=W
            ),
            res[:],
        )


# ---------------- host side ----------------

_CACHE = {}


def _build_cached():
    if "nc" not in _CACHE:
        _CACHE["nc"] = _Builder().build()
    return _CACHE["nc"]


def shard_inputs(x, raw):
    """Build the 8 per-core input dicts from full inputs."""
    in_maps = []
    yb = np.zeros((P, 2), np.float32)
    yb[:, 0] = -np.arange(P, dtype=np.float32)
    yb[:, 1] = (H - 1) - np.arange(P, dtype=np.float32)
    xb = np.zeros((P, 2, BLK), np.float32)
    xs = np.arange(W, dtype=np.float32)
    xb[:, 0, PAD : PAD + W] = -xs
    xb[:, 1, PAD : PAD + W] = (W - 1) - xs
    for core in range(N_CORES):
        b, q = core // 4, core % 4
        z0 = 32 * q
        gz = np.clip(z0 - 6 + np.arange(NB), 0, D - 1)
        rawv = np.zeros((3, P, NB, BLK), np.float32)
        rawv[:, :, :, PAD : PAD + W] = raw[b, 0:3][:, gz].transpose(0, 2, 1, 3)
        rawv = rawv.reshape(3, P, NB * BLK)
        gzx = np.clip(z0 - 2 + np.arange(NXM), 0, D - 1)
        xm = np.zeros((P, NXM, BLK), np.float32)
        xm[:, :, PAD : PAD + W] = x[b, 1][gzx].transpose(1, 0, 2)
        xm = xm.reshape(P, NXM * BLK)
        rawt = np.ascontiguousarray(
            raw[b, 3, z0 : z0 + NOUT].transpose(1, 0, 2).reshape(P, NOUT * W)
        )
        zb = np.zeros((P, 2, 64), np.float32)
        zgl = z0 - 6 + np.arange(NB)
        fake = (zgl < 0) | (zgl > D - 1)
        lo = np.where(fake, 0.0, -zgl.astype(np.float64)).astype(np.float32)
        hi = np.where(fake, 0.0, (D - 1) - zgl.astype(np.float64)).astype(np.float32)
        zb[:, 0, :NB] = lo
        zb[:, 1, :NB] = hi
        in_maps.append(
            {"rawv": rawv, "xm": xm, "rawt": rawt, "zb": zb, "yb": yb, "xb": xb}
        )
    return in_maps


def kernel(x, raw):
    x = np.asarray(x, dtype=np.float32)
    raw = np.asarray(raw, dtype=np.float32)
    nc = _build_cached()
    in_maps = shard_inputs(x, raw)
    from concourse.bass_utils import run_bass_kernel_spmd

    res = run_bass_kernel_spmd(nc, in_maps, core_ids=list(range(N_CORES)))
    out = np.empty((B, 1, D, H, WW), np.float32)
    for core in range(N_CORES):
        b, q = core // 4, core % 4
        z0 = 32 * q
        out[b, 0, z0 : z0 + NOUT] = (
            res.results[core]["out"].reshape(P, NOUT, W).transpose(1, 0, 2)
        )
    return out

